# revision 35
# baseline (speedup 1.0000x reference)
"""CRF integration (nn_CRFIntegrationModule) Trainium2 kernel.

One image per NeuronCore (B=8 -> 8 cores). Each direction's 32-step windowed
scan is computed as a single hardware tensor_tensor_scan on a pre-corrected
input (windowed linear recurrence):

    A32[n] = (A32[n-1] + u'[n-1]) * t[n-1]
    u'[m]  = u[m] - T32[m] * u[m -/+ 32]      (T32 = windowed transfer product)

T32 factors come from two shared doubling chains: M = win-prod of mask (on
GPSIMD) and W = win-sum of plog (on DVE), with T = M * exp(+/-W).  All
elementwise algebra runs in bf16 (DVE 2x mode); scans are mode-less so their
bf16 operands cost the same as fp32.  H results stay in SBUF (no DRAM round
trip); V results are transposed back and fused with the H planes + final
blend in row-major layout.
"""
import os
import sys

for _p in ("/opt/trn_rl_repo", "/root/.axon_site/_ro/trn_rl_repo"):
    if os.path.isdir(_p) and _p not in sys.path:
        sys.path.insert(0, _p)
        break

import numpy as np
import concourse.bacc as bacc
import concourse.mybir as mybir
import concourse.tile as tile
from concourse import masks
from concourse.bass_utils import run_bass_kernel_spmd

Alu = mybir.AluOpType
ActF = mybir.ActivationFunctionType
F32 = mybir.dt.float32
I32 = mybir.dt.int32
BF16 = mybir.dt.bfloat16

B, H, W = 8, 352, 1216
R = 32          # MAXRANGE
CLIP = 5.0      # CLIPVARIANCE
EM5 = float(np.exp(-CLIP))
PAD = 32

# H-phase geometry: row segments (partitions = rows)
RSEGS = [(0, 128), (128, 128), (256, 96)]          # (row0, height)
FH = W + 2 * PAD + 8                               # 1288

# V-phase geometry: transposed layout, 2 chunks x 5 col-segments of <=128 cols
VSEG = H + PAD                                     # 384 per col-seg span
NCS = 5
FV = PAD + NCS * VSEG + 8                          # 1960
VCHUNKS = [(0, 640), (640, 576)]


def _chain(eng, op, dst, t, s1, s2, F):
    """dst[n] = OP_{j=1..32} t[n-j] (left window) via doubling, on engine."""
    tt = eng.tensor_tensor
    tt(s1[:, 2:F], t[:, 1:F - 1], t[:, 0:F - 2], op=op)
    tt(s2[:, 4:F], s1[:, 4:F], s1[:, 2:F - 2], op=op)
    tt(s1[:, 8:F], s2[:, 8:F], s2[:, 4:F - 4], op=op)
    tt(s2[:, 16:F], s1[:, 16:F], s1[:, 8:F - 8], op=op)
    tt(dst[:, 32:F], s2[:, 32:F], s2[:, 16:F - 16], op=op)


def _pad_memsets(nc, t, lo, hi, F, vgaps=False):
    """Zero the pad strips of a [128, F] tile (head, tail, V inter-seg gaps)."""
    g_ = nc.gpsimd
    g_.memset(t[:, 0:lo], 0.0)
    g_.memset(t[:, hi:F], 0.0)
    if vgaps:
        g = t[:, PAD:PAD + 4 * VSEG].rearrange("p (s c) -> p s c", s=4)
        g_.memset(g[:, :, H:VSEG], 0.0)


def _axis_pair(nc, m, p, E0, E1, D, u0, u1, Mw, Sm, c1, c2,
               Pp, eN, eP, u0p, u1p, E0p, E1p, lo, hi, F, awd_out, aw_out):
    """Both directions of one axis.  All tiles bf16 [128, F] except Pp (f32);
    pads zero.  Path exponents are factored out via the global plog prefix P:
    awd = e^{P} * scan(u*e^{-P}, m), so every scan transition is the exact
    bf16 mask and no exp factors compound along paths.
    Scan outputs alias u0/u1/E0/E1.  awd_out/aw_out: destination APs."""
    v = nc.vector
    g = nc.gpsimd
    sc = v.tensor_tensor_scan
    # windowed mask product M[n] = prod_{j=1..32} m[n-j] via a cumulative sum
    # (DVE scan, f32) + windowed difference and ==32 compare (on GPSIMD,
    # hidden under the DVE stream); M needed on [lo, hi+33)
    sc(Sm[:, 1:hi + 33], m[:, 0:hi + 32], m[:, 0:hi + 32], 0.0,
       op0=Alu.add, op1=Alu.bypass)
    g.tensor_tensor(c1[:, lo:hi + 33], Sm[:, lo:hi + 33],
                    Sm[:, lo - R:hi + 33 - R], op=Alu.subtract)
    g.tensor_scalar(Mw[:, lo:hi + 33], c1[:, lo:hi + 33], float(R) - 0.5,
                    None, op0=Alu.is_ge)
    # plog prefix scan P and its exponentials
    sc(Pp[:, lo:hi], p[:, lo - 1:hi - 1], p[:, lo - 1:hi - 1], 0.0,
       op0=Alu.add, op1=Alu.bypass)
    nc.scalar.activation(eN[:, lo:hi], Pp[:, lo:hi], ActF.Exp, scale=-1.0)
    nc.scalar.activation(eP[:, lo:hi], Pp[:, lo:hi], ActF.Exp)
    # u-hat = E * D * e^{-P}  (D scaled in place)
    v.tensor_mul(D[:, lo:hi], D[:, lo:hi], eN[:, lo:hi])
    v.tensor_mul(u0[:, lo:hi], E0[:, lo:hi], D[:, lo:hi])
    v.tensor_mul(u1[:, lo:hi], E1[:, lo:hi], D[:, lo:hi])
    # windowed pre-corrections  u'[m] = u[m] - M * u[m -/+ 32]  (mask-only)
    v.tensor_mul(c1[:, lo:hi], Mw[:, lo:hi], u0[:, lo - R:hi - R])
    v.tensor_sub(u0p[:, lo:hi], u0[:, lo:hi], c1[:, lo:hi])
    v.tensor_mul(c2[:, lo:hi], Mw[:, lo + R + 1:hi + R + 1],
                 u1[:, lo + R:hi + R])
    v.tensor_sub(u1p[:, lo:hi], u1[:, lo:hi], c2[:, lo:hi])
    v.tensor_mul(c1[:, lo:hi], Mw[:, lo:hi], E0[:, lo - R:hi - R])
    v.tensor_sub(E0p[:, lo:hi], E0[:, lo:hi], c1[:, lo:hi])
    v.tensor_mul(c2[:, lo:hi], Mw[:, lo + R + 1:hi + R + 1],
                 E1[:, lo + R:hi + R])
    v.tensor_sub(E1p[:, lo:hi], E1[:, lo:hi], c2[:, lo:hi])
    # windowed scans, all with exact mask transitions (outs alias u0/u1/E0/E1)
    AL, AR, BL, BR = u0, u1, E0, E1
    sc(AL[:, lo:hi], u0p[:, lo - 1:hi - 1], m[:, lo - 1:hi - 1], 0.0,
       op0=Alu.add, op1=Alu.mult)
    sc(AR[:, lo:hi][:, ::-1], u1p[:, lo + 1:hi + 1][:, ::-1],
       m[:, lo + 1:hi + 1][:, ::-1], 0.0, op0=Alu.add, op1=Alu.mult)
    sc(BL[:, lo:hi], E0p[:, lo - 1:hi - 1], m[:, lo - 1:hi - 1], 0.0,
       op0=Alu.add, op1=Alu.mult)
    sc(BR[:, lo:hi][:, ::-1], E1p[:, lo + 1:hi + 1][:, ::-1],
       m[:, lo + 1:hi + 1][:, ::-1], 0.0, op0=Alu.add, op1=Alu.mult)
    v.tensor_add(c1[:, lo:hi], AL[:, lo:hi], AR[:, lo:hi])
    v.tensor_mul(awd_out, c1[:, lo:hi], eP[:, lo:hi])
    v.tensor_add(aw_out, BL[:, lo:hi], BR[:, lo:hi])


def build_program():
    nc = bacc.Bacc("TRN2", target_bir_lowering=False, debug=False)

    pred_log = nc.dram_tensor("pred_log", [2, H, W], F32, kind="ExternalInput").ap()
    mask = nc.dram_tensor("mask", [1, H, W], I32, kind="ExternalInput").ap()
    variance = nc.dram_tensor("variance", [4, H, W], F32, kind="ExternalInput").ap()
    depth_cur = nc.dram_tensor("depth_cur", [1, H, W], F32, kind="ExternalInput").ap()
    depth_orig = nc.dram_tensor("depth_orig", [1, H, W], F32, kind="ExternalInput").ap()
    lam = nc.dram_tensor("lam", [1], F32, kind="ExternalInput").ap()
    depthout = nc.dram_tensor("depthout", [1, H, W], F32, kind="ExternalOutput").ap()

    with tile.TileContext(nc, pool_alloc_mode="queue") as tc:
        with tc.tile_pool(name="const", bufs=1) as cp, \
             tc.tile_pool(name="persist", bufs=1) as ps, \
             tc.tile_pool(name="psum", bufs=8, space="PSUM") as pp:
            identb = cp.tile([128, 128], BF16, tag="identb")
            masks.make_identity(nc, identb[:])
            identf = cp.tile([128, 128], F32, tag="identf")
            masks.make_identity(nc, identf[:])
            lam_t = cp.tile([128, 1], F32, tag="lam")
            nc.sync.dma_start(lam_t[:, 0:1], lam.partition_broadcast(128))

            # persistent row-major planes [128, 3*W] (partition = row-in-seg)
            twH = ps.tile([128, 3 * W], BF16, tag="twH")
            twdH = ps.tile([128, 3 * W], BF16, tag="twdH")
            twT = ps.tile([128, 3 * W], BF16, tag="twT")    # H+V totals
            twdT = ps.tile([128, 3 * W], BF16, tag="twdT")
            mH = ps.tile([128, 3 * W], BF16, tag="mH")
            DoF = ps.tile([128, 3 * W], F32, tag="DoF")
            outO = ps.tile([128, 3 * W], F32, tag="outO")

            _h_phase(nc, tc, pred_log, mask, variance, depth_cur, depth_orig,
                     twH, twdH, mH, DoF)
            bl = dict(mH=mH, DoF=DoF, twT=twT, twdT=twdT,
                      lam_t=lam_t, outO=outO, depthout=depthout)
            _v_phase(nc, tc, pp, identb, identf, pred_log, mask, variance,
                     depth_cur, twH, twdH, twT, twdT, bl)
    nc.finalize()
    return nc


def _h_phase(nc, tc, pred_log, mask, variance, depth, depth_orig,
             twH, twdH, mH, DoF):
    v = nc.vector
    lo, hi = PAD, PAD + W
    with tc.tile_pool(name="hp", bufs=1) as hp:
        def t_(tag, w=FH, dt=BF16, bufs=1):
            return hp.tile([128, w], dt, tag=tag, name=tag, bufs=bufs)

        # single-buffer scratch (produced+consumed inside one segment's DVE
        # stream); pads zeroed once so scan-edge reads stay finite
        c1, c2 = t_("c1"), t_("c2")
        u0p, u1p = t_("u0p"), t_("u1p")
        E0p, E1p = t_("E0p"), t_("E1p")
        for t in (u0p, u1p, E0p, E1p, c1, c2):
            nc.vector.memset(t[:], 0.0)
        for _b in range(2):
            for tg, w, dt in (("m", FH, BF16), ("p", FH, BF16), ("D", FH, BF16),
                              ("Mw", FH, BF16), ("eN", FH, BF16),
                              ("eP", FH, BF16), ("E01", 2 * FH, BF16),
                              ("u01", 2 * FH, BF16), ("Sm", FH, F32)):
                nc.vector.memset(hp.tile([128, w], dt, tag=tg, name=tg,
                                         bufs=2)[:], 0.0)

        for si, (r0, hs) in enumerate(RSEGS):
            rs = slice(r0, r0 + hs)
            # double-buffered per-segment tiles: seg k+1 loads/chains/exps
            # overlap seg k's DVE compute
            m, p, D = (t_("m", bufs=2), t_("p", bufs=2), t_("D", bufs=2))
            Mw = t_("Mw", bufs=2)
            Sm = t_("Sm", FH, F32, bufs=2)
            eN, eP = t_("eN", bufs=2), t_("eP", bufs=2)
            Pp = t_("Pp", FH, F32, bufs=2)
            E01 = t_("E01", 2 * FH, bufs=2)
            u01 = t_("u01", 2 * FH, bufs=2)
            v01 = t_("v01", 2 * FH, F32, bufs=2)
            E0, E1 = E01[:, 0:FH], E01[:, FH:2 * FH]
            u0, u1 = u01[:, 0:FH], u01[:, FH:2 * FH]


            # cast loads (SWDGE): i32/f32 -> bf16 in flight
            nc.gpsimd.dma_start(m[0:hs, lo:hi], mask[0, rs, :])
            nc.gpsimd.dma_start(p[0:hs, lo:hi], pred_log[0, rs, :])
            nc.gpsimd.dma_start(D[0:hs, lo:hi], depth[0, rs, :])
            nc.sync.dma_start(
                v01[0:hs, 0:2 * FH].rearrange("p (s c) -> p s c", s=2)[:, :, lo:hi],
                variance[0:2, rs, :].rearrange("s r c -> r s c"))
            nc.sync.dma_start(DoF[0:hs, si * W:(si + 1) * W],
                              depth_orig[0, rs, :])
            # E = max(exp(-v), e^-5)
            nc.scalar.activation(
                E01[0:hs, 0:2 * FH].rearrange("p (s c) -> p s c", s=2)[:, :, lo:hi],
                v01[0:hs, 0:2 * FH].rearrange("p (s c) -> p s c", s=2)[:, :, lo:hi],
                ActF.Exp, scale=-1.0)
            nc.gpsimd.tensor_scalar_max(
                E01[:, 0:2 * FH].rearrange("p (s c) -> p s c", s=2)[:, :, lo:hi],
                E01[:, 0:2 * FH].rearrange("p (s c) -> p s c", s=2)[:, :, lo:hi],
                EM5)
            v.tensor_copy(mH[:, si * W:(si + 1) * W], m[:, lo:hi])

            _axis_pair(nc, m, p, E0, E1, D, u0, u1, Mw, Sm,
                       c1, c2, Pp, eN, eP, u0p, u1p, E0p, E1p, lo, hi, FH,
                       twdH[:, si * W:(si + 1) * W],
                       twH[:, si * W:(si + 1) * W])


def _tpose_in(nc, pp, ident, stag, dst, cw, c0, mode):
    """Row-major staging [128, 3*640-ish] -> transposed dst [128, FV].
    mode: 'copy_act' | 'copy_dve' | 'exp' (exp applies Exp(-x) in the
    PSUM->SBUF move)."""
    ncs = (cw + 127) // 128
    pdt = stag.dtype

    def emit(d, s):
        if mode == "exp":
            nc.scalar.activation(d, s, ActF.Exp, scale=-1.0)
        elif mode == "copy_act":
            nc.scalar.copy(d, s)
        else:
            nc.vector.tensor_copy(d, s)

    for rp, (r0, hs) in enumerate(RSEGS):
        cs = 0
        while cs < ncs:
            bw = min(128, cw - cs * 128)
            fb = PAD + cs * VSEG + rp * 128
            ng = 0
            while (cs + ng < ncs and ng < 4
                   and min(128, cw - (cs + ng) * 128) == 128):
                ng += 1
            if ng >= 2:
                psu = pp.tile([128, 128 * ng], pdt, tag="pt2b" if pdt == BF16 else "pt2f",
                              bufs=3 if pdt == BF16 else 2, name="psg")
                for g in range(ng):
                    c = 640 * rp + (cs + g) * 128
                    nc.tensor.transpose(psu[:, 128 * g:128 * g + hs],
                                        stag[0:hs, c:c + 128],
                                        ident[0:hs, 0:hs])
                src = psu[:, 0:128 * ng].rearrange(
                    "p (s c) -> p s c", s=ng)[:, :, 0:hs]
                d = dst[:, fb:fb + VSEG * (ng - 1) + VSEG].rearrange(
                    "p (s c) -> p s c", s=ng)[:, :, 0:hs]
                emit(d, src)
                cs += ng
            else:
                psu = pp.tile([128, 128], pdt, tag="ptb" if pdt == BF16 else "ptf",
                              bufs=2 if pdt == BF16 else 1)
                c = 640 * rp + cs * 128
                nc.tensor.transpose(psu[0:bw, 0:hs], stag[0:hs, c:c + bw],
                                    ident[0:hs, 0:hs])
                emit(dst[0:bw, fb:fb + hs], psu[0:bw, 0:hs])
                cs += 1


def _tpose_out_acc(nc, pp, ident, src, hsrc, dst, cw, c0):
    """Transposed src [128, FV] bf16 -> row-major: dst = src^T + hsrc.
    One PSUM-operand tensor_tensor add per merged group."""
    v = nc.vector
    ncs = (cw + 127) // 128
    for rp, (r0, hs) in enumerate(RSEGS):
        cs = 0
        while cs < ncs:
            bw = min(128, cw - cs * 128)
            fb = PAD + cs * VSEG + rp * 128
            ng = 0
            while (cs + ng < ncs and ng < 4
                   and min(128, cw - (cs + ng) * 128) == 128):
                ng += 1
            cb = rp * W + c0 + cs * 128
            if ng >= 2:
                psu = pp.tile([128, 128 * ng], BF16, tag="pt2b", bufs=3,
                              name="psg")
                for g in range(ng):
                    nc.tensor.transpose(
                        psu[0:hs, 128 * g:128 * (g + 1)],
                        src[:, fb + VSEG * g:fb + VSEG * g + hs],
                        ident[:, :])
                v.tensor_tensor(dst[0:hs, cb:cb + 128 * ng],
                                psu[0:hs, 0:128 * ng],
                                hsrc[0:hs, cb:cb + 128 * ng], op=Alu.add)
                cs += ng
            else:
                psu = pp.tile([128, 128], BF16, tag="ptb", bufs=2)
                nc.tensor.transpose(psu[0:hs, 0:bw], src[0:bw, fb:fb + hs],
                                    ident[0:bw, 0:bw])
                v.tensor_tensor(dst[0:hs, cb:cb + bw], psu[0:hs, 0:bw],
                                hsrc[0:hs, cb:cb + bw], op=Alu.add)
                cs += 1


def _stage_load(nc, stag, dram_plane, c0, cw, gp=False):
    """DRAM [H, W] cols [c0,c0+cw) -> staging [128, (seg,640)] row-major."""
    eng = nc.gpsimd if gp else nc.sync
    eng.dma_start(
        stag[:, 0:2 * 640].rearrange("p (s c) -> p s c", s=2)[:, :, 0:cw],
        dram_plane[0:256, c0:c0 + cw].rearrange("(s p) c -> p s c", p=128))
    eng.dma_start(stag[0:96, 2 * 640:2 * 640 + cw],
                  dram_plane[256:352, c0:c0 + cw])


def _v_phase(nc, tc, pp, identb, identf, pred_log, mask, variance, depth,
             twH, twdH, twT, twdT, bl):
    v = nc.vector
    lo = PAD
    vhi = PAD + (NCS - 1) * VSEG + H      # 1920
    with tc.tile_pool(name="vp", bufs=1) as vp:
        def t_(tag, dt=BF16, bufs=1):
            return vp.tile([128, FV], dt, tag=tag, name=tag, bufs=bufs)

        u0, u1 = t_("vu0"), t_("vu1")
        c1, c2 = t_("vc1"), t_("vc2")
        bl["selB"] = vp.tile([128, 3 * W], BF16, tag="selB", name="selB")
        bl["rcpB"] = vp.tile([128, 3 * W], BF16, tag="rcpB", name="rcpB")
        bl["nwB"] = vp.tile([128, 3 * W], BF16, tag="nwB", name="nwB")
        Pp = t_("vPp", F32)
        eN, eP = t_("veN"), t_("veP")
        u0p, u1p = t_("vu0p"), t_("vu1p")
        E0p, E1p = t_("vE0p"), t_("vE1p")
        for t in (u0, u1, u0p, u1p, E0p, E1p, c1, c2, eN, eP):
            nc.vector.memset(t[:], 0.0)
        nc.vector.memset(Pp[:], 0.0)
        # both buffers of the double-buffered inputs: DVE, at phase start
        for _b in range(2):
            for tg in ("vm", "vp_", "vD", "vE0", "vE1"):
                nc.vector.memset(vp.tile([128, FV], BF16, tag=tg, name=tg,
                                         bufs=2)[:], 0.0)
        nc.vector.memset(vp.tile([128, FV], BF16, tag="vMw",
                                 name="vMw")[:], 0.0)
        nc.vector.memset(vp.tile([128, FV], F32, tag="vSm",
                                 name="vSm")[:], 0.0)

        with tc.tile_pool(name="vstage", bufs=1) as sp:
            for ci, (c0, cw) in enumerate(VCHUNKS):
                ncs = (cw + 127) // 128
                hi = PAD + (ncs - 1) * VSEG + H
                m, p = t_("vm", bufs=2), t_("vp_", bufs=2)
                D = t_("vD", bufs=2)
                E0, E1 = t_("vE0", bufs=2), t_("vE1", bufs=2)
                Mw = t_("vMw")
                Sm = vp.tile([128, FV], F32, tag="vSm", name="vSm")


                sb1 = sp.tile([128, 3 * 640], BF16, tag="sb1", bufs=1)
                sb2 = sp.tile([128, 3 * 640], BF16, tag="sb2", bufs=1)
                sf1 = sp.tile([128, 3 * 640], F32, tag="sf1", bufs=2)
                _stage_load(nc, sb1, mask[0], c0, cw, gp=True)
                _tpose_in(nc, pp, identb, sb1, m, cw, c0, "copy_act")
                _stage_load(nc, sb2, pred_log[1], c0, cw, gp=True)
                _tpose_in(nc, pp, identb, sb2, p, cw, c0, "copy_act")
                _stage_load(nc, sf1, variance[2], c0, cw)
                _tpose_in(nc, pp, identf, sf1, E0, cw, c0, "exp")
                sf2 = sp.tile([128, 3 * 640], F32, tag="sf1", bufs=2)
                _stage_load(nc, sf2, variance[3], c0, cw)
                _tpose_in(nc, pp, identf, sf2, E1, cw, c0, "exp")
                sf3 = sp.tile([128, 3 * 640], F32, tag="sf1", bufs=2)
                _stage_load(nc, sf3, depth[0], c0, cw)
                _tpose_in(nc, pp, identf, sf3, D, cw, c0, "copy_act")

                # stale cols when cw isn't a multiple of 128 (chunk 1: 64-wide
                # last col-seg): zero partitions [bw,128) of that segment span
                lbw = cw - (ncs - 1) * 128
                if lbw < 128:
                    fb = PAD + (ncs - 1) * VSEG
                    for t in (m, p):
                        v.memset(t[lbw:128, fb:fb + H], 0.0)

                nc.gpsimd.tensor_scalar_max(E0[:, lo:hi], E0[:, lo:hi], EM5)
                nc.gpsimd.tensor_scalar_max(E1[:, lo:hi], E1[:, lo:hi], EM5)

                _axis_pair(nc, m, p, E0, E1, D, u0, u1, Mw, Sm,
                           c1, c2, Pp, eN, eP, u0p, u1p, E0p, E1p, lo, hi, FV,
                           c1[:, lo:hi], c2[:, lo:hi])
                # c1 = awd_V, c2 = aw_V (transposed); add H planes on the out
                _tpose_out_acc(nc, pp, identb, c1, twdH, twdT, cw, c0)
                _tpose_out_acc(nc, pp, identb, c2, twH, twT, cw, c0)
                _blend_chunk(nc, bl, c0, cw)


def _blend_chunk(nc, bl, c0, cw):
    """Final blend for V-chunk columns [c0, c0+cw) on row-major planes,
    via [128, 3, cw] strided views of the [128, 3*W] tiles."""
    v = nc.vector

    def cs(t):
        return t[:, 0:3 * W].rearrange("p (s c) -> p s c", s=3)[:, :, c0:c0 + cw]

    mH, DoF, twT, twdT = bl["mH"], bl["DoF"], bl["twT"], bl["twdT"]
    selB, rcpB, nwB = bl["selB"], bl["rcpB"], bl["nwB"]
    outO, lam_t = bl["outO"], bl["lam_t"]
    v.tensor_scalar(cs(selB), cs(twT), 0.0, None, op0=Alu.is_gt)
    v.tensor_mul(cs(selB), cs(selB), cs(mH))
    nc.scalar.activation(cs(selB), cs(selB), ActF.Copy, scale=lam_t[:, 0:1])
    v.tensor_scalar_max(cs(twT), cs(twT), 1e-6)
    nc.scalar.activation(cs(outO), cs(twT), ActF.Ln)
    nc.scalar.activation(cs(rcpB), cs(outO), ActF.Exp, scale=-1.0)
    # one Newton step: r1 = r0 * (2 - tw * r0)
    v.tensor_mul(cs(nwB), cs(twT), cs(rcpB))
    nc.scalar.activation(cs(nwB), cs(nwB), ActF.Copy, bias=2.0, scale=-1.0)
    v.tensor_mul(cs(rcpB), cs(rcpB), cs(nwB))
    v.tensor_mul(cs(twdT), cs(twdT), cs(rcpB))       # lat = twd / tw
    v.tensor_sub(cs(twdT), cs(twdT), cs(DoF))        # lat - Do (mixed dtype)
    v.tensor_mul(cs(twdT), cs(twdT), cs(selB))       # * sel * lam
    v.tensor_tensor(cs(outO), cs(DoF), cs(twdT), op=Alu.add)
    for si, (r0, hs) in enumerate(RSEGS):
        rs = slice(r0, r0 + hs)
        nc.sync.dma_start(bl["depthout"][0, rs, c0:c0 + cw],
                          bl["outO"][0:hs, si * W + c0:si * W + c0 + cw])


_NC = None


def _get_nc():
    global _NC
    if _NC is None:
        _NC = build_program()
    return _NC


def kernel(pred_log, mask, variance, depthin, lam, times):
    pred_log = np.ascontiguousarray(np.asarray(pred_log, dtype=np.float32))
    mask = np.ascontiguousarray(np.asarray(mask, dtype=np.int32))
    variance = np.ascontiguousarray(np.asarray(variance, dtype=np.float32))
    depthin = np.ascontiguousarray(np.asarray(depthin, dtype=np.float32))
    lam = np.ascontiguousarray(np.asarray(lam, dtype=np.float32)).reshape(1)
    t = int(np.asarray(times))

    if t <= 0:
        return depthin.copy()
    nc = _get_nc()
    depth_cur = depthin
    for _ in range(t):
        in_maps = [{
            "pred_log": pred_log[b],
            "mask": mask[b],
            "variance": variance[b],
            "depth_cur": depth_cur[b],
            "depth_orig": depthin[b],
            "lam": lam,
        } for b in range(B)]
        res = run_bass_kernel_spmd(nc, in_maps, list(range(B)))
        depth_cur = np.stack([res.results[i]["depthout"] for i in range(B)])
    return depth_cur.astype(np.float32)


# revision 36
# speedup vs baseline: 1.2532x; 1.2532x over previous
"""CRF integration (nn_CRFIntegrationModule) Trainium2 kernel.

One image per NeuronCore (B=8 -> 8 cores). Each direction's 32-step windowed
scan is computed as a single hardware tensor_tensor_scan on a pre-corrected
input (windowed linear recurrence):

    A32[n] = (A32[n-1] + u'[n-1]) * t[n-1]
    u'[m]  = u[m] - T32[m] * u[m -/+ 32]      (T32 = windowed transfer product)

T32 factors come from two shared doubling chains: M = win-prod of mask (on
GPSIMD) and W = win-sum of plog (on DVE), with T = M * exp(+/-W).  All
elementwise algebra runs in bf16 (DVE 2x mode); scans are mode-less so their
bf16 operands cost the same as fp32.  H results stay in SBUF (no DRAM round
trip); V results are transposed back and fused with the H planes + final
blend in row-major layout.
"""
import os
import sys

for _p in ("/opt/trn_rl_repo", "/root/.axon_site/_ro/trn_rl_repo"):
    if os.path.isdir(_p) and _p not in sys.path:
        sys.path.insert(0, _p)
        break

import numpy as np
import concourse.bacc as bacc
import concourse.mybir as mybir
import concourse.tile as tile
from concourse import masks
from concourse.bass_utils import run_bass_kernel_spmd

Alu = mybir.AluOpType
ActF = mybir.ActivationFunctionType
F32 = mybir.dt.float32
I32 = mybir.dt.int32
BF16 = mybir.dt.bfloat16

B, H, W = 8, 352, 1216
R = 32          # MAXRANGE
CLIP = 5.0      # CLIPVARIANCE
EM5 = float(np.exp(-CLIP))
PAD = 32

# H-phase geometry: row segments (partitions = rows)
RSEGS = [(0, 128), (128, 128), (256, 96)]          # (row0, height)
FH = W + 2 * PAD + 8                               # 1288

# V-phase geometry: transposed layout, 2 chunks x 5 col-segments of <=128 cols
VSEG = H + PAD                                     # 384 per col-seg span
NCS = 5
FV = PAD + NCS * VSEG + 8                          # 1960
VCHUNKS = [(0, 640), (640, 576)]


def _chain(eng, op, dst, t, s1, s2, F):
    """dst[n] = OP_{j=1..32} t[n-j] (left window) via doubling, on engine."""
    tt = eng.tensor_tensor
    tt(s1[:, 2:F], t[:, 1:F - 1], t[:, 0:F - 2], op=op)
    tt(s2[:, 4:F], s1[:, 4:F], s1[:, 2:F - 2], op=op)
    tt(s1[:, 8:F], s2[:, 8:F], s2[:, 4:F - 4], op=op)
    tt(s2[:, 16:F], s1[:, 16:F], s1[:, 8:F - 8], op=op)
    tt(dst[:, 32:F], s2[:, 32:F], s2[:, 16:F - 16], op=op)


def _pad_memsets(nc, t, lo, hi, F, vgaps=False):
    """Zero the pad strips of a [128, F] tile (head, tail, V inter-seg gaps)."""
    g_ = nc.gpsimd
    g_.memset(t[:, 0:lo], 0.0)
    g_.memset(t[:, hi:F], 0.0)
    if vgaps:
        g = t[:, PAD:PAD + 4 * VSEG].rearrange("p (s c) -> p s c", s=4)
        g_.memset(g[:, :, H:VSEG], 0.0)


def _axis_pair(nc, m, p, E0, E1, D, u0, u1, Mw, Sm, c1, c2,
               Pp, eN, eP, u0p, u1p, E0p, E1p, lo, hi, F, awd_out, aw_out):
    """Both directions of one axis.  All tiles bf16 [128, F] except Pp (f32);
    pads zero.  Path exponents are factored out via the global plog prefix P:
    awd = e^{P} * scan(u*e^{-P}, m), so every scan transition is the exact
    bf16 mask and no exp factors compound along paths.
    Scan outputs alias u0/u1/E0/E1.  awd_out/aw_out: destination APs."""
    v = nc.vector
    g = nc.gpsimd
    sc = v.tensor_tensor_scan
    # windowed mask product M[n] = prod_{j=1..32} m[n-j] via a cumulative sum
    # (DVE scan, f32) + windowed difference and ==32 compare (on GPSIMD,
    # hidden under the DVE stream); M needed on [lo, hi+33)
    sc(Sm[:, 1:hi + 33], m[:, 0:hi + 32], m[:, 0:hi + 32], 0.0,
       op0=Alu.add, op1=Alu.bypass)
    g.tensor_tensor(c1[:, lo:hi + 33], Sm[:, lo:hi + 33],
                    Sm[:, lo - R:hi + 33 - R], op=Alu.subtract)
    g.tensor_scalar(Mw[:, lo:hi + 33], c1[:, lo:hi + 33], float(R) - 0.5,
                    None, op0=Alu.is_ge)
    # plog prefix scan P and its exponentials
    sc(Pp[:, lo:hi], p[:, lo - 1:hi - 1], p[:, lo - 1:hi - 1], 0.0,
       op0=Alu.add, op1=Alu.bypass)
    nc.scalar.activation(eN[:, lo:hi], Pp[:, lo:hi], ActF.Exp, scale=-1.0)
    nc.scalar.activation(eP[:, lo:hi], Pp[:, lo:hi], ActF.Exp)
    # u-hat = E * D * e^{-P}  (D scaled in place)
    v.tensor_mul(D[:, lo:hi], D[:, lo:hi], eN[:, lo:hi])
    v.tensor_mul(u0[:, lo:hi], E0[:, lo:hi], D[:, lo:hi])
    v.tensor_mul(u1[:, lo:hi], E1[:, lo:hi], D[:, lo:hi])
    # windowed pre-corrections  u'[m] = u[m] - M * u[m -/+ 32]  (mask-only)
    v.tensor_mul(c1[:, lo:hi], Mw[:, lo:hi], u0[:, lo - R:hi - R])
    v.tensor_sub(u0p[:, lo:hi], u0[:, lo:hi], c1[:, lo:hi])
    v.tensor_mul(c2[:, lo:hi], Mw[:, lo + R + 1:hi + R + 1],
                 u1[:, lo + R:hi + R])
    v.tensor_sub(u1p[:, lo:hi], u1[:, lo:hi], c2[:, lo:hi])
    v.tensor_mul(c1[:, lo:hi], Mw[:, lo:hi], E0[:, lo - R:hi - R])
    v.tensor_sub(E0p[:, lo:hi], E0[:, lo:hi], c1[:, lo:hi])
    v.tensor_mul(c2[:, lo:hi], Mw[:, lo + R + 1:hi + R + 1],
                 E1[:, lo + R:hi + R])
    v.tensor_sub(E1p[:, lo:hi], E1[:, lo:hi], c2[:, lo:hi])
    # windowed scans, all with exact mask transitions (outs alias u0/u1/E0/E1)
    AL, AR, BL, BR = u0, u1, E0, E1
    sc(AL[:, lo:hi], u0p[:, lo - 1:hi - 1], m[:, lo - 1:hi - 1], 0.0,
       op0=Alu.add, op1=Alu.mult)
    sc(AR[:, lo:hi][:, ::-1], u1p[:, lo + 1:hi + 1][:, ::-1],
       m[:, lo + 1:hi + 1][:, ::-1], 0.0, op0=Alu.add, op1=Alu.mult)
    sc(BL[:, lo:hi], E0p[:, lo - 1:hi - 1], m[:, lo - 1:hi - 1], 0.0,
       op0=Alu.add, op1=Alu.mult)
    sc(BR[:, lo:hi][:, ::-1], E1p[:, lo + 1:hi + 1][:, ::-1],
       m[:, lo + 1:hi + 1][:, ::-1], 0.0, op0=Alu.add, op1=Alu.mult)
    v.tensor_add(c1[:, lo:hi], AL[:, lo:hi], AR[:, lo:hi])
    v.tensor_mul(awd_out, c1[:, lo:hi], eP[:, lo:hi])
    v.tensor_add(aw_out, BL[:, lo:hi], BR[:, lo:hi])


def build_program():
    nc = bacc.Bacc("TRN2", target_bir_lowering=False, debug=False)

    pred_log = nc.dram_tensor("pred_log", [2, H, W], F32, kind="ExternalInput").ap()
    mask = nc.dram_tensor("mask", [1, H, W], I32, kind="ExternalInput").ap()
    variance = nc.dram_tensor("variance", [4, H, W], F32, kind="ExternalInput").ap()
    depth_cur = nc.dram_tensor("depth_cur", [1, H, W], F32, kind="ExternalInput").ap()
    depth_orig = nc.dram_tensor("depth_orig", [1, H, W], F32, kind="ExternalInput").ap()
    lam = nc.dram_tensor("lam", [1], F32, kind="ExternalInput").ap()
    depthout = nc.dram_tensor("depthout", [1, H, W], F32, kind="ExternalOutput").ap()

    with tile.TileContext(nc, pool_alloc_mode="queue") as tc:
        with tc.tile_pool(name="const", bufs=1) as cp, \
             tc.tile_pool(name="persist", bufs=1) as ps, \
             tc.tile_pool(name="psum", bufs=8, space="PSUM") as pp:
            identb = cp.tile([128, 128], BF16, tag="identb")
            masks.make_identity(nc, identb[:])
            identf = cp.tile([128, 128], F32, tag="identf")
            masks.make_identity(nc, identf[:])
            lam_t = cp.tile([128, 1], F32, tag="lam")
            nc.sync.dma_start(lam_t[:, 0:1], lam.partition_broadcast(128))

            # persistent row-major planes [128, 3*W] (partition = row-in-seg)
            twH = ps.tile([128, 3 * W], BF16, tag="twH")
            twdH = ps.tile([128, 3 * W], BF16, tag="twdH")
            twT = ps.tile([128, 3 * W], BF16, tag="twT")    # H+V totals
            twdT = ps.tile([128, 3 * W], BF16, tag="twdT")
            mH = ps.tile([128, 3 * W], BF16, tag="mH")
            DoF = ps.tile([128, 3 * W], F32, tag="DoF")
            outO = ps.tile([128, 3 * W], F32, tag="outO")

            _h_phase(nc, tc, pred_log, mask, variance, depth_cur, depth_orig,
                     twH, twdH, mH, DoF)
            bl = dict(mH=mH, DoF=DoF, twT=twT, twdT=twdT,
                      lam_t=lam_t, outO=outO, depthout=depthout)
            _v_phase(nc, tc, pp, identb, identf, pred_log, mask, variance,
                     depth_cur, twH, twdH, twT, twdT, bl)
    nc.finalize()
    return nc


def _h_phase(nc, tc, pred_log, mask, variance, depth, depth_orig,
             twH, twdH, mH, DoF):
    v = nc.vector
    lo, hi = PAD, PAD + W
    with tc.tile_pool(name="hp", bufs=1) as hp:
        def t_(tag, w=FH, dt=BF16, bufs=1):
            return hp.tile([128, w], dt, tag=tag, name=tag, bufs=bufs)

        # single-buffer scratch (produced+consumed inside one segment's DVE
        # stream); pads zeroed once so scan-edge reads stay finite
        c1, c2 = t_("c1"), t_("c2")
        u0p, u1p = t_("u0p"), t_("u1p")
        E0p, E1p = t_("E0p"), t_("E1p")
        for t in (u0p, u1p, E0p, E1p):
            _pad_memsets(nc, t, lo, hi, FH)

        for si, (r0, hs) in enumerate(RSEGS):
            rs = slice(r0, r0 + hs)
            # double-buffered per-segment tiles: seg k+1 loads/chains/exps
            # overlap seg k's DVE compute
            m, p, D = (t_("m", bufs=2), t_("p", bufs=2), t_("D", bufs=2))
            Mw = t_("Mw", bufs=2)
            Sm = t_("Sm", FH, F32, bufs=2)
            eN, eP = t_("eN", bufs=2), t_("eP", bufs=2)
            Pp = t_("Pp", FH, F32, bufs=2)
            E01 = t_("E01", 2 * FH, bufs=2)
            u01 = t_("u01", 2 * FH, bufs=2)
            v01 = t_("v01", 2 * FH, F32, bufs=2)
            E0, E1 = E01[:, 0:FH], E01[:, FH:2 * FH]
            u0, u1 = u01[:, 0:FH], u01[:, FH:2 * FH]


            # cast loads (SWDGE): i32/f32 -> bf16 in flight
            nc.gpsimd.dma_start(m[0:hs, lo:hi], mask[0, rs, :])
            nc.gpsimd.dma_start(p[0:hs, lo:hi], pred_log[0, rs, :])
            nc.gpsimd.dma_start(D[0:hs, lo:hi], depth[0, rs, :])
            nc.sync.dma_start(
                v01[0:hs, 0:2 * FH].rearrange("p (s c) -> p s c", s=2)[:, :, lo:hi],
                variance[0:2, rs, :].rearrange("s r c -> r s c"))
            nc.sync.dma_start(DoF[0:hs, si * W:(si + 1) * W],
                              depth_orig[0, rs, :])
            # E = max(exp(-v), e^-5)
            nc.scalar.activation(
                E01[0:hs, 0:2 * FH].rearrange("p (s c) -> p s c", s=2)[:, :, lo:hi],
                v01[0:hs, 0:2 * FH].rearrange("p (s c) -> p s c", s=2)[:, :, lo:hi],
                ActF.Exp, scale=-1.0)
            nc.gpsimd.tensor_scalar_max(
                E01[:, 0:2 * FH].rearrange("p (s c) -> p s c", s=2)[:, :, lo:hi],
                E01[:, 0:2 * FH].rearrange("p (s c) -> p s c", s=2)[:, :, lo:hi],
                EM5)
            v.tensor_copy(mH[:, si * W:(si + 1) * W], m[:, lo:hi])

            _axis_pair(nc, m, p, E0, E1, D, u0, u1, Mw, Sm,
                       c1, c2, Pp, eN, eP, u0p, u1p, E0p, E1p, lo, hi, FH,
                       twdH[:, si * W:(si + 1) * W],
                       twH[:, si * W:(si + 1) * W])


def _tpose_in(nc, pp, ident, stag, dst, cw, c0, mode):
    """Row-major staging [128, 3*640-ish] -> transposed dst [128, FV].
    mode: 'copy_act' | 'copy_dve' | 'exp' (exp applies Exp(-x) in the
    PSUM->SBUF move)."""
    ncs = (cw + 127) // 128
    pdt = stag.dtype

    def emit(d, s):
        if mode == "exp":
            nc.scalar.activation(d, s, ActF.Exp, scale=-1.0)
        elif mode == "copy_act":
            nc.scalar.copy(d, s)
        else:
            nc.vector.tensor_copy(d, s)

    for rp, (r0, hs) in enumerate(RSEGS):
        cs = 0
        while cs < ncs:
            bw = min(128, cw - cs * 128)
            fb = PAD + cs * VSEG + rp * 128
            ng = 0
            while (cs + ng < ncs and ng < 4
                   and min(128, cw - (cs + ng) * 128) == 128):
                ng += 1
            if ng >= 2:
                psu = pp.tile([128, 128 * ng], pdt, tag="pt2b" if pdt == BF16 else "pt2f",
                              bufs=3 if pdt == BF16 else 2, name="psg")
                for g in range(ng):
                    c = 640 * rp + (cs + g) * 128
                    nc.tensor.transpose(psu[:, 128 * g:128 * g + hs],
                                        stag[0:hs, c:c + 128],
                                        ident[0:hs, 0:hs])
                src = psu[:, 0:128 * ng].rearrange(
                    "p (s c) -> p s c", s=ng)[:, :, 0:hs]
                d = dst[:, fb:fb + VSEG * (ng - 1) + VSEG].rearrange(
                    "p (s c) -> p s c", s=ng)[:, :, 0:hs]
                emit(d, src)
                cs += ng
            else:
                psu = pp.tile([128, 128], pdt, tag="ptb" if pdt == BF16 else "ptf",
                              bufs=2 if pdt == BF16 else 1)
                c = 640 * rp + cs * 128
                nc.tensor.transpose(psu[0:bw, 0:hs], stag[0:hs, c:c + bw],
                                    ident[0:hs, 0:hs])
                emit(dst[0:bw, fb:fb + hs], psu[0:bw, 0:hs])
                cs += 1


def _tpose_out_acc(nc, pp, ident, src, hsrc, dst, cw, c0):
    """Transposed src [128, FV] bf16 -> row-major: dst = src^T + hsrc.
    One PSUM-operand tensor_tensor add per merged group."""
    v = nc.vector
    ncs = (cw + 127) // 128
    for rp, (r0, hs) in enumerate(RSEGS):
        cs = 0
        while cs < ncs:
            bw = min(128, cw - cs * 128)
            fb = PAD + cs * VSEG + rp * 128
            ng = 0
            while (cs + ng < ncs and ng < 4
                   and min(128, cw - (cs + ng) * 128) == 128):
                ng += 1
            cb = rp * W + c0 + cs * 128
            if ng >= 2:
                psu = pp.tile([128, 128 * ng], BF16, tag="pt2b", bufs=3,
                              name="psg")
                for g in range(ng):
                    nc.tensor.transpose(
                        psu[0:hs, 128 * g:128 * (g + 1)],
                        src[:, fb + VSEG * g:fb + VSEG * g + hs],
                        ident[:, :])
                v.tensor_tensor(dst[0:hs, cb:cb + 128 * ng],
                                psu[0:hs, 0:128 * ng],
                                hsrc[0:hs, cb:cb + 128 * ng], op=Alu.add)
                cs += ng
            else:
                psu = pp.tile([128, 128], BF16, tag="ptb", bufs=2)
                nc.tensor.transpose(psu[0:hs, 0:bw], src[0:bw, fb:fb + hs],
                                    ident[0:bw, 0:bw])
                v.tensor_tensor(dst[0:hs, cb:cb + bw], psu[0:hs, 0:bw],
                                hsrc[0:hs, cb:cb + bw], op=Alu.add)
                cs += 1


def _stage_load(nc, stag, dram_plane, c0, cw, gp=False):
    """DRAM [H, W] cols [c0,c0+cw) -> staging [128, (seg,640)] row-major."""
    eng = nc.gpsimd if gp else nc.sync
    eng.dma_start(
        stag[:, 0:2 * 640].rearrange("p (s c) -> p s c", s=2)[:, :, 0:cw],
        dram_plane[0:256, c0:c0 + cw].rearrange("(s p) c -> p s c", p=128))
    eng.dma_start(stag[0:96, 2 * 640:2 * 640 + cw],
                  dram_plane[256:352, c0:c0 + cw])


def _v_phase(nc, tc, pp, identb, identf, pred_log, mask, variance, depth,
             twH, twdH, twT, twdT, bl):
    v = nc.vector
    lo = PAD
    vhi = PAD + (NCS - 1) * VSEG + H      # 1920
    with tc.tile_pool(name="vp", bufs=1) as vp:
        def t_(tag, dt=BF16, bufs=1):
            return vp.tile([128, FV], dt, tag=tag, name=tag, bufs=bufs)

        u0, u1 = t_("vu0"), t_("vu1")
        c1, c2 = t_("vc1"), t_("vc2")
        bl["selB"] = vp.tile([128, 3 * W], BF16, tag="selB", name="selB")
        bl["rcpB"] = vp.tile([128, 3 * W], BF16, tag="rcpB", name="rcpB")
        bl["nwB"] = vp.tile([128, 3 * W], BF16, tag="nwB", name="nwB")
        Pp = t_("vPp", F32)
        eN, eP = t_("veN"), t_("veP")
        u0p, u1p = t_("vu0p"), t_("vu1p")
        E0p, E1p = t_("vE0p"), t_("vE1p")
        for t in (u0, u1, u0p, u1p, E0p, E1p):
            _pad_memsets(nc, t, lo, vhi, FV)

        with tc.tile_pool(name="vstage", bufs=1) as sp:
            for ci, (c0, cw) in enumerate(VCHUNKS):
                ncs = (cw + 127) // 128
                hi = PAD + (ncs - 1) * VSEG + H
                m, p = t_("vm", bufs=2), t_("vp_", bufs=2)
                D = t_("vD", bufs=2)
                E0, E1 = t_("vE0", bufs=2), t_("vE1", bufs=2)
                Mw = t_("vMw")
                Sm = vp.tile([128, FV], F32, tag="vSm", name="vSm")


                if ci < 2:
                    for t in (m, p, D, E0, E1):
                        _pad_memsets(nc, t, lo, vhi, FV, vgaps=True)
                if ci < 1:
                    nc.vector.memset(Sm[:, 0:1], 0.0)
                sb1 = sp.tile([128, 3 * 640], BF16, tag="sb1", bufs=1)
                sb2 = sp.tile([128, 3 * 640], BF16, tag="sb2", bufs=1)
                sf1 = sp.tile([128, 3 * 640], F32, tag="sf1", bufs=2)
                _stage_load(nc, sb1, mask[0], c0, cw, gp=True)
                _tpose_in(nc, pp, identb, sb1, m, cw, c0, "copy_act")
                _stage_load(nc, sb2, pred_log[1], c0, cw, gp=True)
                _tpose_in(nc, pp, identb, sb2, p, cw, c0, "copy_act")
                _stage_load(nc, sf1, variance[2], c0, cw)
                _tpose_in(nc, pp, identf, sf1, E0, cw, c0, "exp")
                sf2 = sp.tile([128, 3 * 640], F32, tag="sf1", bufs=2)
                _stage_load(nc, sf2, variance[3], c0, cw)
                _tpose_in(nc, pp, identf, sf2, E1, cw, c0, "exp")
                sf3 = sp.tile([128, 3 * 640], F32, tag="sf1", bufs=2)
                _stage_load(nc, sf3, depth[0], c0, cw)
                _tpose_in(nc, pp, identf, sf3, D, cw, c0, "copy_act")

                # stale cols when cw isn't a multiple of 128 (chunk 1: 64-wide
                # last col-seg): zero partitions [bw,128) of that segment span
                lbw = cw - (ncs - 1) * 128
                if lbw < 128:
                    fb = PAD + (ncs - 1) * VSEG
                    for t in (m, p, D, E0, E1):
                        v.memset(t[lbw:128, fb:fb + H], 0.0)

                nc.gpsimd.tensor_scalar_max(E0[:, lo:hi], E0[:, lo:hi], EM5)
                nc.gpsimd.tensor_scalar_max(E1[:, lo:hi], E1[:, lo:hi], EM5)

                _axis_pair(nc, m, p, E0, E1, D, u0, u1, Mw, Sm,
                           c1, c2, Pp, eN, eP, u0p, u1p, E0p, E1p, lo, hi, FV,
                           c1[:, lo:hi], c2[:, lo:hi])
                # c1 = awd_V, c2 = aw_V (transposed); add H planes on the out
                _tpose_out_acc(nc, pp, identb, c1, twdH, twdT, cw, c0)
                _tpose_out_acc(nc, pp, identb, c2, twH, twT, cw, c0)
                _blend_chunk(nc, bl, c0, cw)


def _blend_chunk(nc, bl, c0, cw):
    """Final blend for V-chunk columns [c0, c0+cw) on row-major planes,
    via [128, 3, cw] strided views of the [128, 3*W] tiles."""
    v = nc.vector

    def cs(t):
        return t[:, 0:3 * W].rearrange("p (s c) -> p s c", s=3)[:, :, c0:c0 + cw]

    mH, DoF, twT, twdT = bl["mH"], bl["DoF"], bl["twT"], bl["twdT"]
    selB, rcpB, nwB = bl["selB"], bl["rcpB"], bl["nwB"]
    outO, lam_t = bl["outO"], bl["lam_t"]
    v.tensor_scalar(cs(selB), cs(twT), 0.0, None, op0=Alu.is_gt)
    v.tensor_mul(cs(selB), cs(selB), cs(mH))
    nc.scalar.activation(cs(selB), cs(selB), ActF.Copy, scale=lam_t[:, 0:1])
    v.tensor_scalar_max(cs(twT), cs(twT), 1e-6)
    nc.scalar.activation(cs(outO), cs(twT), ActF.Ln)
    nc.scalar.activation(cs(rcpB), cs(outO), ActF.Exp, scale=-1.0)
    # one Newton step: r1 = r0 * (2 - tw * r0)
    v.tensor_mul(cs(nwB), cs(twT), cs(rcpB))
    nc.scalar.activation(cs(nwB), cs(nwB), ActF.Copy, bias=2.0, scale=-1.0)
    v.tensor_mul(cs(rcpB), cs(rcpB), cs(nwB))
    v.tensor_mul(cs(twdT), cs(twdT), cs(rcpB))       # lat = twd / tw
    v.tensor_sub(cs(twdT), cs(twdT), cs(DoF))        # lat - Do (mixed dtype)
    v.tensor_mul(cs(twdT), cs(twdT), cs(selB))       # * sel * lam
    v.tensor_tensor(cs(outO), cs(DoF), cs(twdT), op=Alu.add)
    for si, (r0, hs) in enumerate(RSEGS):
        rs = slice(r0, r0 + hs)
        nc.sync.dma_start(bl["depthout"][0, rs, c0:c0 + cw],
                          bl["outO"][0:hs, si * W + c0:si * W + c0 + cw])


_NC = None


def _get_nc():
    global _NC
    if _NC is None:
        _NC = build_program()
    return _NC


def kernel(pred_log, mask, variance, depthin, lam, times):
    pred_log = np.ascontiguousarray(np.asarray(pred_log, dtype=np.float32))
    mask = np.ascontiguousarray(np.asarray(mask, dtype=np.int32))
    variance = np.ascontiguousarray(np.asarray(variance, dtype=np.float32))
    depthin = np.ascontiguousarray(np.asarray(depthin, dtype=np.float32))
    lam = np.ascontiguousarray(np.asarray(lam, dtype=np.float32)).reshape(1)
    t = int(np.asarray(times))

    if t <= 0:
        return depthin.copy()
    nc = _get_nc()
    depth_cur = depthin
    for _ in range(t):
        in_maps = [{
            "pred_log": pred_log[b],
            "mask": mask[b],
            "variance": variance[b],
            "depth_cur": depth_cur[b],
            "depth_orig": depthin[b],
            "lam": lam,
        } for b in range(B)]
        res = run_bass_kernel_spmd(nc, in_maps, list(range(B)))
        depth_cur = np.stack([res.results[i]["depthout"] for i in range(B)])
    return depth_cur.astype(np.float32)
